# revision 32
# baseline (speedup 1.0000x reference)
"""BoltzmannRouter Trainium2 kernel: 8-core data-parallel Bass implementation.

Full inputs: x (4, 4096, 2048) f32, gate_w (64, 2048) f32.
Output: routing weights (4, 4096, 64) f32 (softmax -> top-44 mask -> renorm).

Sharding: 16384 tokens split 2048/core across 8 NeuronCores; gate weight
replicated.

Final design (DMA/Vector/PE co-bound, ~8.75MB DMA, ~25us DVE per core):
  - x AND gate_w ship as plain fp16: halves DMA bytes and matmul passes vs
    the fp16x3 baseline, and the 64-wide stationary halves PE array energy
    (the chip power-throttles, so energy is wall-clock). Simulated rel err
    6.2e-3 vs the 2e-2 gate.
  - x is packed host-side so every DMA descriptor line is 4KB (4 contraction
    chunks x 512 tokens of one 128-row block), keeping all 16 DMA engines
    busy while arriving in token-group order for pipelining.
  - softmax max-subtraction and the +eps term are dropped (|s|<=3 so exp is
    in [0.05, 25]; eps*S/ws < 1e-6 relative).
  - scores copy to SBUF as fp16 (PSUM can't feed PE stationaries); the
    negate + token-transpose fold into one fp16 matmul against -I; the
    1/W_SCALE descale folds into the exp activation scale.
  - engine split: Scalar computes u=exp(s) and u2=exp(-s) (GPSIMD cannot
    touch PSUM); GpSimd makes the destructible top-k copy and runs
    normalize_recip; Vector keeps only max8/match_replace + the masked
    multiply-accumulate.
  - per-group emission order (phase then selection) so each group's
    selection starts right after its own transposes; the scalar copy of the
    NEXT group overlaps that group's matmuls.
  - software-pipelined emission: selection(g-1) is emitted after group g's
    matmul/copy/transpose phase so the in-order PE and Scalar streams run
    ahead at DMA pace; output DMAs sit at the very end of the SP stream so
    no x dispatch ever queues behind an output wait.
  - token groups of 256/512/512/512/256 shrink the pipeline head and tail.

Measured: ~56.6-58.6us HW exec (vs 76.2us fp16x3 baseline), rel err 6.2e-3.
Note: the chip power-throttles under sustained load (util limit 0.4-0.6 in
NTFF counters), inflating per-op costs; scheduling variants beyond this
structure measured equal-or-worse within that noise band.
"""

import os
import sys

sys.path.insert(0, "/opt/trn_rl_repo")

import numpy as np

D = 2048
E = 64
N_BOTTOM = 20  # 64 experts - 44 active
NEG_BIG = -1e30
TEMPERATURE = 2.718281828459045
N_CORES = 8
TPC = 2048  # tokens per core
GROUPS = (256, 512, 512, 512, 256)  # token groups per core (sum = TPC)
KC = 16  # contraction chunks of 128
CPL = 4  # chunks packed per DMA line
JB = KC // CPL  # row-blocks in the packed x layout
GMAX = max(GROUPS)
N_SUB_TOT = TPC // 128

W_SCALE = 64.0  # 2^6: lifts gate_w into fp16-normal range
LO_SCALE = 4096.0  # 2^12: scale on the low fp16 split part of gate_w


def _build_nc():
    import concourse.bacc as bacc
    import concourse.mybir as mybir
    from concourse.tile import TileContext

    F32 = mybir.dt.float32
    F16 = mybir.dt.float16

    lean_tail = os.environ.get("BOLTZ_LEAN_TAIL", "1") == "1"
    if lean_tail:
        # the stock Tile exit emits drain + barrier + sem-clear + barrier
        # (~8us); the kernel preamble already range-clears the semaphores at
        # the start of every execution, so drain + one barrier suffices
        def _lean_drain_and_barrier(self, tick_clock, wait_clock):
            from concourse.tile import ScopedClock

            drain_inst = self.nc.sync.drain()
            wait_clock.add_sem_waits(
                drain_inst.ins, ScopedClock({None: tick_clock.global_clock})
            )
            self.nc.all_engine_barrier()
            popped = self.nc._tile_sem_poison_stack.pop()
            assert popped is self._sem_poison
            self.sems.allocated()

        TileContext._drain_and_barrier = _lean_drain_and_barrier

    nc = bacc.Bacc(None, target_bir_lowering=False)
    # packed x: row (j*128+p), col layout per group g: [c, t] blocks at 4*t0
    xpk_d = nc.declare_dram_parameter("xpk", [JB * 128, CPL * TPC], F16, isOutput=False)
    whl_d = nc.declare_dram_parameter("whl", [D, E], F16, isOutput=False)
    out_d = nc.declare_dram_parameter("out", [TPC, E], F16, isOutput=True)

    with TileContext(nc) as tc:
        with (
            tc.tile_pool(name="const", bufs=1) as cpool,
            tc.tile_pool(name="xg", bufs=3) as xpool,
            tc.tile_pool(name="ssb", bufs=2) as spool,
            tc.tile_pool(name="og", bufs=len(GROUPS)) as opool,
            tc.tile_pool(name="work", bufs=4) as wkpool,
            tc.tile_pool(name="small", bufs=6) as smpool,
            tc.tile_pool(name="ps_s", bufs=2, space="PSUM") as ps_pool,
            tc.tile_pool(name="ps_t", bufs=4, space="PSUM") as pst_pool,
        ):
            # m_sb = [-I | +I] (f16): one matmul by it transposes scores
            # to token-major with both signs side by side. Built on-device
            # (memset + two diagonal fills) so no DMA dispatch sits ahead
            # of the x stream on the SP queue.
            m_sb = cpool.tile([E, 2 * E], F16)
            nc.gpsimd.memset(m_sb, 0.0)
            nc.gpsimd.affine_select(
                out=m_sb[:, :E],
                in_=m_sb[:, :E],
                compare_op=mybir.AluOpType.not_equal,
                fill=-1.0,
                base=0,
                pattern=[[-1, E]],
                channel_multiplier=1,
            )
            nc.gpsimd.affine_select(
                out=m_sb[:, E:],
                in_=m_sb[:, E:],
                compare_op=mybir.AluOpType.not_equal,
                fill=1.0,
                base=0,
                pattern=[[-1, E]],
                channel_multiplier=1,
            )
            whl_sb = cpool.tile([128, KC, E], F16)
            nc.sync.dma_start(
                out=whl_sb, in_=whl_d[:, :].rearrange("(kc p) e -> p kc e", p=128)
            )
            # x DMAs in group order: all but the last dispatch up-front on
            # the SP queue; the last one issues from the DVE queue mid-way
            # (DVE is idle early; SP dispatches cost ~1.3us each throttled)
            xpk_r = xpk_d[:, :].rearrange("(j p) x -> p j x", p=128)
            xgs = []
            t0 = 0
            for gi, L in enumerate(GROUPS):
                xg = xpool.tile([128, JB, CPL, L], F16, tag=f"xg{L}")
                nc.sync.dma_start(out=xg, in_=xpk_r[:, :, CPL * t0 : CPL * (t0 + L)])
                xgs.append((t0, L, xg))
                t0 += L

            def emit_mms(t0, L, xg):
                # scores: ps rows 0:64 = wh.T@xh (x W_SCALE), 64:128 = wl.T@xh
                ps_full = ps_pool.tile([E, GMAX], F32, tag="ps")
                ps = ps_full[:, :L]
                for kc in range(KC):
                    nc.tensor.matmul(
                        ps,
                        lhsT=whl_sb[:, kc, :],
                        rhs=xg[:, kc // CPL, kc % CPL, :],
                        start=(kc == 0),
                        stop=(kc == KC - 1),
                    )
                return ps

            def emit_copy_transpose(L, ps):
                n_sub = L // 128
                # PSUM -> SBUF (f16) so PE can re-read it as a stationary
                ps_sb_full = spool.tile([E, GMAX], F16, tag="ssb")
                ps_sb = ps_sb_full[:, :L]
                nc.scalar.copy(ps_sb, ps)
                psum_ts = []
                for s in range(n_sub):
                    # token-major scaled scores, both signs: [-s | +s] x W
                    psum_t = pst_pool.tile([128, 2 * E], F32, tag="ps_t")
                    nc.tensor.matmul(
                        psum_t, lhsT=ps_sb[:, s * 128 : (s + 1) * 128], rhs=m_sb
                    )
                    psum_ts.append(psum_t)
                return psum_ts

            out_dmas = []

            def emit_selection(t0, L, psum_ts):
                n_sub = L // 128
                og = opool.tile([128, n_sub, E], F16, tag=f"og_{t0}")
                for s in range(n_sub):
                    psum_t = psum_ts[s]
                    # one exp over [-s|+s] gives u2=exp(-s) and u=exp(s)
                    # side by side; 1/W_SCALE descale folded into the scale
                    u2u = wkpool.tile([128, 2 * E], F32, tag="u2u")
                    nc.scalar.activation(
                        u2u,
                        psum_t,
                        mybir.ActivationFunctionType.Exp,
                        scale=1.0 / W_SCALE,
                    )
                    u2 = u2u[:, :E]
                    u = u2u[:, E:]

                    # destructible copy of u2 for the top-k chain (SBUF-only)
                    y = wkpool.tile([128, E], F32, tag="y")
                    nc.gpsimd.tensor_copy(y, u2)

                    # largest-20 u2 = bottom-20 scores. 2x(max8+replace)
                    # removes 16, then ranks 17-24 -> index 4 = 21st largest
                    # u2 = 21st smallest score = boundary kept expert.
                    r1 = smpool.tile([128, 8], F32, tag="r1")
                    nc.vector.max(r1, y)
                    nc.vector.match_replace(y, r1, y, NEG_BIG)
                    r2 = smpool.tile([128, 8], F32, tag="r2")
                    nc.vector.max(r2, y)
                    nc.vector.match_replace(y, r2, y, NEG_BIG)
                    r3 = smpool.tile([128, 8], F32, tag="r3")
                    nc.vector.max(r3, y)
                    thr2 = r3[:, (N_BOTTOM - 16) : (N_BOTTOM - 16 + 1)]

                    # wm = u * (u2 <= thr2); ws = sum(wm)
                    wm = wkpool.tile([128, E], F32, tag="wm")
                    ws = smpool.tile([128, 1], F32, tag="ws")
                    nc.vector.scalar_tensor_tensor(
                        out=wm,
                        in0=u2,
                        scalar=thr2,
                        in1=u,
                        op0=mybir.AluOpType.is_le,
                        op1=mybir.AluOpType.mult,
                        accum_out=ws,
                    )
                    # og = wm/ws (f16 cast on write); ws -> 1/ws, unused
                    nc.gpsimd.normalize_recip(og[:, s, :], wm, ws)
                out_dmas.append((t0, L, og))

            # software pipeline: selection(g-1) emitted after phase(g) so
            # the in-order PE/Scalar phase streams run ahead at DMA pace
            prev = None
            for gi, (t0, L, xg) in enumerate(xgs):
                ps = emit_mms(t0, L, xg)
                psum_ts = emit_copy_transpose(L, ps)
                if prev is not None:
                    emit_selection(*prev)
                prev = (t0, L, psum_ts)
            emit_selection(*prev)

            # all output DMAs at the very end of the SP stream so no x
            # dispatch ever queues behind an output wait
            for t0, L, og in out_dmas:
                nc.sync.dma_start(
                    out=out_d[t0 : t0 + L, :].rearrange("(s p) e -> p s e", p=128),
                    in_=og,
                )

    nc.finalize()
    return nc


_NC = None
LAST_EXEC_NS = None
LAST_RESULTS = None


def _get_nc():
    global _NC
    if _NC is None:
        _NC = _build_nc()
    return _NC


def _pack_x(shard_t_f16):
    """[D, TPC] f16 -> [JB*128, CPL*TPC] with 4KB-contiguous group lines."""
    x4 = shard_t_f16.reshape(JB, CPL, 128, TPC)  # [j, c, p, t]
    blocks = []
    t0 = 0
    for L in GROUPS:
        blocks.append(
            x4[:, :, :, t0 : t0 + L].transpose(0, 2, 1, 3).reshape(JB, 128, CPL * L)
        )
        t0 += L
    return np.ascontiguousarray(
        np.concatenate(blocks, axis=2).reshape(JB * 128, CPL * TPC)
    )


def kernel(x, gate_w, trace=False):
    global LAST_EXEC_NS, LAST_RESULTS
    from concourse.bass_utils import run_bass_kernel_spmd

    x = np.asarray(x)
    gate_w = np.asarray(gate_w)
    Btot = x.shape[0] * x.shape[1]
    x2 = x.reshape(Btot, D).astype(np.float32, copy=False)

    wt = np.ascontiguousarray(
        gate_w.astype(np.float32, copy=False).T * np.float32(W_SCALE / TEMPERATURE)
    )
    whl = np.ascontiguousarray(wt.astype(np.float16))

    nc = _get_nc()
    in_maps = []
    for i in range(N_CORES):
        shard_t = np.ascontiguousarray(
            x2[i * TPC : (i + 1) * TPC].T.astype(np.float16)
        )
        in_maps.append({"xpk": _pack_x(shard_t), "whl": whl})

    kwargs = {}
    if trace:
        try:
            import antenv.axon_hooks  # noqa: F401  (shimmed by test harness)

            kwargs["trace"] = True
        except ImportError:
            pass
    res = run_bass_kernel_spmd(nc, in_maps, core_ids=list(range(N_CORES)), **kwargs)
    LAST_EXEC_NS = res.exec_time_ns
    LAST_RESULTS = res
    out = np.concatenate(
        [res.results[i]["out"].astype(np.float32) for i in range(N_CORES)], axis=0
    )
    return out.reshape(x.shape[0], x.shape[1], E)


# revision 33
# speedup vs baseline: 1.0122x; 1.0122x over previous
"""BoltzmannRouter Trainium2 kernel: 8-core data-parallel Bass implementation.

Full inputs: x (4, 4096, 2048) f32, gate_w (64, 2048) f32.
Output: routing weights (4, 4096, 64) f32 (softmax -> top-44 mask -> renorm).

Sharding: 16384 tokens split 2048/core across 8 NeuronCores; gate weight
replicated.

Final design (DMA/Vector/PE co-bound, ~8.75MB DMA, ~25us DVE per core):
  - x AND gate_w ship as plain fp16: halves DMA bytes and matmul passes vs
    the fp16x3 baseline, and the 64-wide stationary halves PE array energy
    (the chip power-throttles, so energy is wall-clock). Simulated rel err
    6.2e-3 vs the 2e-2 gate.
  - x is packed host-side so every DMA descriptor line is 4KB (4 contraction
    chunks x 512 tokens of one 128-row block), keeping all 16 DMA engines
    busy while arriving in token-group order for pipelining.
  - softmax max-subtraction and the +eps term are dropped (|s|<=3 so exp is
    in [0.05, 25]; eps*S/ws < 1e-6 relative).
  - scores copy to SBUF as fp16 (PSUM can't feed PE stationaries); the
    negate + token-transpose fold into one fp16 matmul against -I; the
    1/W_SCALE descale folds into the exp activation scale.
  - engine split: Scalar computes u=exp(s) and u2=exp(-s) (GPSIMD cannot
    touch PSUM); GpSimd makes the destructible top-k copy and runs
    normalize_recip; Vector keeps only max8/match_replace + the masked
    multiply-accumulate.
  - per-group emission order (phase then selection) so each group's
    selection starts right after its own transposes; the scalar copy of the
    NEXT group overlaps that group's matmuls.
  - software-pipelined emission: selection(g-1) is emitted after group g's
    matmul/copy/transpose phase so the in-order PE and Scalar streams run
    ahead at DMA pace; output DMAs sit at the very end of the SP stream so
    no x dispatch ever queues behind an output wait.
  - token groups of 256/512/512/512/256 shrink the pipeline head and tail.

Measured: ~56.6-58.6us HW exec (vs 76.2us fp16x3 baseline), rel err 6.2e-3.
Note: the chip power-throttles under sustained load (util limit 0.4-0.6 in
NTFF counters), inflating per-op costs; scheduling variants beyond this
structure measured equal-or-worse within that noise band.
"""

import os
import sys

sys.path.insert(0, "/opt/trn_rl_repo")

import numpy as np

D = 2048
E = 64
N_BOTTOM = 20  # 64 experts - 44 active
NEG_BIG = -1e30
TEMPERATURE = 2.718281828459045
N_CORES = 8
TPC = 2048  # tokens per core
GROUPS = (256, 512, 512, 512, 256)  # token groups per core (sum = TPC)
KC = 16  # contraction chunks of 128
CPL = 4  # chunks packed per DMA line
JB = KC // CPL  # row-blocks in the packed x layout
GMAX = max(GROUPS)
N_SUB_TOT = TPC // 128

W_SCALE = 64.0  # 2^6: lifts gate_w into fp16-normal range
LO_SCALE = 4096.0  # 2^12: scale on the low fp16 split part of gate_w


def _build_nc():
    import concourse.bacc as bacc
    import concourse.mybir as mybir
    from concourse.tile import TileContext

    F32 = mybir.dt.float32
    F16 = mybir.dt.float16

    lean_tail = os.environ.get("BOLTZ_LEAN_TAIL", "1") == "1"
    if lean_tail:
        # the stock Tile exit emits drain + barrier + sem-clear + barrier
        # (~8us); the kernel preamble already range-clears the semaphores at
        # the start of every execution, so drain + one barrier suffices
        def _lean_drain_and_barrier(self, tick_clock, wait_clock):
            from concourse.tile import ScopedClock

            drain_inst = self.nc.sync.drain()
            wait_clock.add_sem_waits(
                drain_inst.ins, ScopedClock({None: tick_clock.global_clock})
            )
            self.nc.all_engine_barrier()
            popped = self.nc._tile_sem_poison_stack.pop()
            assert popped is self._sem_poison
            self.sems.allocated()

        TileContext._drain_and_barrier = _lean_drain_and_barrier

    nc = bacc.Bacc(None, target_bir_lowering=False)
    # packed x: row (j*128+p), col layout per group g: [c, t] blocks at 4*t0
    xpk_d = nc.declare_dram_parameter("xpk", [JB * 128, CPL * TPC], F16, isOutput=False)
    whl_d = nc.declare_dram_parameter("whl", [D, E], F16, isOutput=False)
    # M' = [[-I], [-I*2^-12]] (f16): one matmul combines hi+lo, negates and
    # transposes scores to token-major; psum_t = -W_SCALE * s
    m_d = nc.declare_dram_parameter("mconst", [E, 2 * E], F16, isOutput=False)
    out_d = nc.declare_dram_parameter("out", [TPC, E], F16, isOutput=True)

    with TileContext(nc) as tc:
        with (
            tc.tile_pool(name="const", bufs=1) as cpool,
            tc.tile_pool(name="xg", bufs=3) as xpool,
            tc.tile_pool(name="ssb", bufs=2) as spool,
            tc.tile_pool(name="og", bufs=len(GROUPS)) as opool,
            tc.tile_pool(name="work", bufs=4) as wkpool,
            tc.tile_pool(name="small", bufs=6) as smpool,
            tc.tile_pool(name="ps_s", bufs=2, space="PSUM") as ps_pool,
            tc.tile_pool(name="ps_t", bufs=4, space="PSUM") as pst_pool,
        ):
            m_sb = cpool.tile([E, 2 * E], F16)
            nc.sync.dma_start(out=m_sb, in_=m_d[:, :])
            whl_sb = cpool.tile([128, KC, E], F16)
            nc.sync.dma_start(
                out=whl_sb, in_=whl_d[:, :].rearrange("(kc p) e -> p kc e", p=128)
            )
            # x DMAs in group order: all but the last dispatch up-front on
            # the SP queue; the last one issues from the DVE queue mid-way
            # (DVE is idle early; SP dispatches cost ~1.3us each throttled)
            xpk_r = xpk_d[:, :].rearrange("(j p) x -> p j x", p=128)
            xgs = []
            t0 = 0
            for gi, L in enumerate(GROUPS):
                xg = xpool.tile([128, JB, CPL, L], F16, tag=f"xg{L}")
                nc.sync.dma_start(out=xg, in_=xpk_r[:, :, CPL * t0 : CPL * (t0 + L)])
                xgs.append((t0, L, xg))
                t0 += L

            def emit_mms(t0, L, xg):
                # scores: ps rows 0:64 = wh.T@xh (x W_SCALE), 64:128 = wl.T@xh
                ps_full = ps_pool.tile([E, GMAX], F32, tag="ps")
                ps = ps_full[:, :L]
                for kc in range(KC):
                    nc.tensor.matmul(
                        ps,
                        lhsT=whl_sb[:, kc, :],
                        rhs=xg[:, kc // CPL, kc % CPL, :],
                        start=(kc == 0),
                        stop=(kc == KC - 1),
                    )
                return ps

            def emit_copy_transpose(L, ps):
                n_sub = L // 128
                # PSUM -> SBUF (f16) so PE can re-read it as a stationary
                ps_sb_full = spool.tile([E, GMAX], F16, tag="ssb")
                ps_sb = ps_sb_full[:, :L]
                nc.scalar.copy(ps_sb, ps)
                psum_ts = []
                for s in range(n_sub):
                    # token-major scaled scores, both signs: [-s | +s] x W
                    psum_t = pst_pool.tile([128, 2 * E], F32, tag="ps_t")
                    nc.tensor.matmul(
                        psum_t, lhsT=ps_sb[:, s * 128 : (s + 1) * 128], rhs=m_sb
                    )
                    psum_ts.append(psum_t)
                return psum_ts

            out_dmas = []

            def emit_selection(t0, L, psum_ts):
                n_sub = L // 128
                og = opool.tile([128, n_sub, E], F16, tag=f"og_{t0}")
                for s in range(n_sub):
                    psum_t = psum_ts[s]
                    # one exp over [-s|+s] gives u2=exp(-s) and u=exp(s)
                    # side by side; 1/W_SCALE descale folded into the scale
                    u2u = wkpool.tile([128, 2 * E], F32, tag="u2u")
                    nc.scalar.activation(
                        u2u,
                        psum_t,
                        mybir.ActivationFunctionType.Exp,
                        scale=1.0 / W_SCALE,
                    )
                    u2 = u2u[:, :E]
                    u = u2u[:, E:]

                    # destructible copy of u2 for the top-k chain (SBUF-only)
                    y = wkpool.tile([128, E], F32, tag="y")
                    nc.gpsimd.tensor_copy(y, u2)

                    # largest-20 u2 = bottom-20 scores. 2x(max8+replace)
                    # removes 16, then ranks 17-24 -> index 4 = 21st largest
                    # u2 = 21st smallest score = boundary kept expert.
                    r1 = smpool.tile([128, 8], F32, tag="r1")
                    nc.vector.max(r1, y)
                    nc.vector.match_replace(y, r1, y, NEG_BIG)
                    r2 = smpool.tile([128, 8], F32, tag="r2")
                    nc.vector.max(r2, y)
                    nc.vector.match_replace(y, r2, y, NEG_BIG)
                    r3 = smpool.tile([128, 8], F32, tag="r3")
                    nc.vector.max(r3, y)
                    thr2 = r3[:, (N_BOTTOM - 16) : (N_BOTTOM - 16 + 1)]

                    # wm = u * (u2 <= thr2); ws = sum(wm)
                    wm = wkpool.tile([128, E], F32, tag="wm")
                    ws = smpool.tile([128, 1], F32, tag="ws")
                    nc.vector.scalar_tensor_tensor(
                        out=wm,
                        in0=u2,
                        scalar=thr2,
                        in1=u,
                        op0=mybir.AluOpType.is_le,
                        op1=mybir.AluOpType.mult,
                        accum_out=ws,
                    )
                    # og = wm/ws (f16 cast on write); ws -> 1/ws, unused
                    nc.gpsimd.normalize_recip(og[:, s, :], wm, ws)
                out_dmas.append((t0, L, og))

            # software pipeline: selection(g-1) emitted after phase(g) so
            # the in-order PE/Scalar phase streams run ahead at DMA pace
            prev = None
            for gi, (t0, L, xg) in enumerate(xgs):
                ps = emit_mms(t0, L, xg)
                psum_ts = emit_copy_transpose(L, ps)
                if prev is not None:
                    emit_selection(*prev)
                prev = (t0, L, psum_ts)
            emit_selection(*prev)

            # all output DMAs at the very end of the SP stream so no x
            # dispatch ever queues behind an output wait
            for t0, L, og in out_dmas:
                nc.sync.dma_start(
                    out=out_d[t0 : t0 + L, :].rearrange("(s p) e -> p s e", p=128),
                    in_=og,
                )

    nc.finalize()
    return nc


_NC = None
LAST_EXEC_NS = None
LAST_RESULTS = None


def _get_nc():
    global _NC
    if _NC is None:
        _NC = _build_nc()
    return _NC


def _pack_x(shard_t_f16):
    """[D, TPC] f16 -> [JB*128, CPL*TPC] with 4KB-contiguous group lines."""
    x4 = shard_t_f16.reshape(JB, CPL, 128, TPC)  # [j, c, p, t]
    blocks = []
    t0 = 0
    for L in GROUPS:
        blocks.append(
            x4[:, :, :, t0 : t0 + L].transpose(0, 2, 1, 3).reshape(JB, 128, CPL * L)
        )
        t0 += L
    return np.ascontiguousarray(
        np.concatenate(blocks, axis=2).reshape(JB * 128, CPL * TPC)
    )


def _make_mconst():
    m = np.zeros((E, 2 * E), np.float16)
    idx = np.arange(E)
    m[idx, idx] = np.float16(-1.0)
    m[idx, E + idx] = np.float16(1.0)
    return m


def kernel(x, gate_w, trace=False):
    global LAST_EXEC_NS, LAST_RESULTS
    from concourse.bass_utils import run_bass_kernel_spmd

    x = np.asarray(x)
    gate_w = np.asarray(gate_w)
    Btot = x.shape[0] * x.shape[1]
    x2 = x.reshape(Btot, D).astype(np.float32, copy=False)

    wt = np.ascontiguousarray(
        gate_w.astype(np.float32, copy=False).T * np.float32(W_SCALE / TEMPERATURE)
    )
    whl = np.ascontiguousarray(wt.astype(np.float16))
    mconst = _make_mconst()

    nc = _get_nc()
    in_maps = []
    for i in range(N_CORES):
        shard_t = np.ascontiguousarray(
            x2[i * TPC : (i + 1) * TPC].T.astype(np.float16)
        )
        in_maps.append({"xpk": _pack_x(shard_t), "whl": whl, "mconst": mconst})

    kwargs = {}
    if trace:
        try:
            import antenv.axon_hooks  # noqa: F401  (shimmed by test harness)

            kwargs["trace"] = True
        except ImportError:
            pass
    res = run_bass_kernel_spmd(nc, in_maps, core_ids=list(range(N_CORES)), **kwargs)
    LAST_EXEC_NS = res.exec_time_ns
    LAST_RESULTS = res
    out = np.concatenate(
        [res.results[i]["out"].astype(np.float32) for i in range(N_CORES)], axis=0
    )
    return out.reshape(x.shape[0], x.shape[1], E)


# revision 35
# speedup vs baseline: 1.2132x; 1.1986x over previous
"""BoltzmannRouter Trainium2 kernel: 8-core data-parallel Bass implementation.

Full inputs: x (4, 4096, 2048) f32, gate_w (64, 2048) f32.
Output: routing weights (4, 4096, 64) f32 (softmax -> top-44 mask -> renorm).

Sharding: 16384 tokens split 2048/core across 8 NeuronCores; gate weight
replicated.

Final design (DMA/Vector/PE co-bound, ~8.75MB DMA, ~25us DVE per core):
  - x AND gate_w ship as plain fp16: halves DMA bytes and matmul passes vs
    the fp16x3 baseline, and the 64-wide stationary halves PE array energy
    (the chip power-throttles, so energy is wall-clock). Simulated rel err
    6.2e-3 vs the 2e-2 gate.
  - x is packed host-side so every DMA descriptor line is 4KB (4 contraction
    chunks x 512 tokens of one 128-row block), keeping all 16 DMA engines
    busy while arriving in token-group order for pipelining.
  - softmax max-subtraction and the +eps term are dropped (|s|<=3 so exp is
    in [0.05, 25]; eps*S/ws < 1e-6 relative).
  - scores copy to SBUF as fp16 (PSUM can't feed PE stationaries); the
    token-transpose matmul uses the constant [-I | +I] so each subtile gets
    [-s | +s] side by side, and ONE Scalar exp activation then yields both
    u2=exp(-s) and u=exp(s) (1/W_SCALE descale folded into the exp scale).
  - engine split: Scalar does the exp (GPSIMD cannot touch PSUM); GpSimd
    makes the destructible top-k copy and runs normalize_recip; Vector
    keeps only max8/match_replace + the masked multiply-accumulate.
  - per-group emission order (phase then selection) so each group's
    selection starts right after its own transposes; the scalar copy of the
    NEXT group overlaps that group's matmuls.
  - software-pipelined emission: selection(g-1) is emitted after group g's
    matmul/copy/transpose phase so the in-order PE and Scalar streams run
    ahead at DMA pace; output DMAs sit at the very end of the SP stream so
    no x dispatch ever queues behind an output wait.
  - token groups of 256/512/512/512/256 shrink the pipeline head and tail.

Measured: ~55.9-57.7us HW exec (vs 76.2us fp16x3 baseline), rel err 6.2e-3.
Note: the chip power-throttles under sustained load (util limit 0.4-0.6 in
NTFF counters), inflating per-op costs; scheduling variants beyond this
structure measured equal-or-worse within that noise band.
"""

import os
import sys

sys.path.insert(0, "/opt/trn_rl_repo")

import numpy as np

D = 2048
E = 64
N_BOTTOM = 20  # 64 experts - 44 active
NEG_BIG = -1e30
TEMPERATURE = 2.718281828459045
N_CORES = 8
TPC = 2048  # tokens per core
GROUPS = (256, 512, 512, 512, 256)  # token groups per core (sum = TPC)
KC = 16  # contraction chunks of 128
CPL = 4  # chunks packed per DMA line
JB = KC // CPL  # row-blocks in the packed x layout
GMAX = max(GROUPS)
N_SUB_TOT = TPC // 128

W_SCALE = 64.0  # 2^6: lifts gate_w into fp16-normal range
LO_SCALE = 4096.0  # 2^12: scale on the low fp16 split part of gate_w


def _build_nc():
    import concourse.bacc as bacc
    import concourse.mybir as mybir
    from concourse.tile import TileContext

    F32 = mybir.dt.float32
    F16 = mybir.dt.float16

    lean_tail = os.environ.get("BOLTZ_LEAN_TAIL", "1") == "1"
    if lean_tail:
        # the stock Tile exit emits drain + barrier + sem-clear + barrier
        # (~8us); the kernel preamble already range-clears the semaphores at
        # the start of every execution, so drain + one barrier suffices
        def _lean_drain_and_barrier(self, tick_clock, wait_clock):
            from concourse.tile import ScopedClock

            drain_inst = self.nc.sync.drain()
            wait_clock.add_sem_waits(
                drain_inst.ins, ScopedClock({None: tick_clock.global_clock})
            )
            self.nc.all_engine_barrier()
            popped = self.nc._tile_sem_poison_stack.pop()
            assert popped is self._sem_poison
            self.sems.allocated()

        TileContext._drain_and_barrier = _lean_drain_and_barrier

    nc = bacc.Bacc(None, target_bir_lowering=False)
    # packed x: row (j*128+p), col layout per group g: [c, t] blocks at 4*t0
    xpk_d = nc.declare_dram_parameter("xpk", [JB * 128, CPL * TPC], F16, isOutput=False)
    whl_d = nc.declare_dram_parameter("whl", [D, E], F16, isOutput=False)
    # M' = [[-I], [-I*2^-12]] (f16): one matmul combines hi+lo, negates and
    # transposes scores to token-major; psum_t = -W_SCALE * s
    m_d = nc.declare_dram_parameter("mconst", [E, 3 * E], F16, isOutput=False)
    out_d = nc.declare_dram_parameter("out", [TPC, E], F16, isOutput=True)

    with TileContext(nc) as tc:
        with (
            tc.tile_pool(name="const", bufs=1) as cpool,
            tc.tile_pool(name="xg", bufs=3) as xpool,
            tc.tile_pool(name="ssb", bufs=2) as spool,
            tc.tile_pool(name="og", bufs=len(GROUPS)) as opool,
            tc.tile_pool(name="work", bufs=4) as wkpool,
            tc.tile_pool(name="small", bufs=6) as smpool,
            tc.tile_pool(name="ps_s", bufs=2, space="PSUM") as ps_pool,
            tc.tile_pool(name="ps_t", bufs=4, space="PSUM") as pst_pool,
        ):
            m_sb = cpool.tile([E, 3 * E], F16)
            nc.sync.dma_start(out=m_sb, in_=m_d[:, :])
            whl_sb = cpool.tile([128, KC, E], F16)
            nc.sync.dma_start(
                out=whl_sb, in_=whl_d[:, :].rearrange("(kc p) e -> p kc e", p=128)
            )
            # x DMAs in group order: all but the last dispatch up-front on
            # the SP queue; the last one issues from the DVE queue mid-way
            # (DVE is idle early; SP dispatches cost ~1.3us each throttled)
            xpk_r = xpk_d[:, :].rearrange("(j p) x -> p j x", p=128)
            xgs = []
            t0 = 0
            for gi, L in enumerate(GROUPS):
                xg = xpool.tile([128, JB, CPL, L], F16, tag=f"xg{L}")
                nc.sync.dma_start(out=xg, in_=xpk_r[:, :, CPL * t0 : CPL * (t0 + L)])
                xgs.append((t0, L, xg))
                t0 += L

            def emit_mms(t0, L, xg):
                # scores: ps rows 0:64 = wh.T@xh (x W_SCALE), 64:128 = wl.T@xh
                ps_full = ps_pool.tile([E, GMAX], F32, tag="ps")
                ps = ps_full[:, :L]
                for kc in range(KC):
                    nc.tensor.matmul(
                        ps,
                        lhsT=whl_sb[:, kc, :],
                        rhs=xg[:, kc // CPL, kc % CPL, :],
                        start=(kc == 0),
                        stop=(kc == KC - 1),
                    )
                return ps

            def emit_copy_transpose(L, ps):
                n_sub = L // 128
                # PSUM -> SBUF (f16) so PE can re-read it as a stationary
                ps_sb_full = spool.tile([E, GMAX], F16, tag="ssb")
                ps_sb = ps_sb_full[:, :L]
                nc.scalar.copy(ps_sb, ps)
                psum_ts = []
                for s in range(n_sub):
                    # token-major scaled scores [-s | +s | -s] x W: the third
                    # copy becomes the destructible top-k buffer after exp
                    psum_t = pst_pool.tile([128, 3 * E], F32, tag="ps_t")
                    nc.tensor.matmul(
                        psum_t, lhsT=ps_sb[:, s * 128 : (s + 1) * 128], rhs=m_sb
                    )
                    psum_ts.append(psum_t)
                return psum_ts

            out_dmas = []

            def emit_selection(t0, L, psum_ts):
                n_sub = L // 128
                og = opool.tile([128, n_sub, E], F16, tag=f"og_{t0}")
                for s in range(n_sub):
                    psum_t = psum_ts[s]
                    # one exp over [-s|+s|-s] gives u2=exp(-s), u=exp(s) and
                    # y = a bitwise-identical second exp(-s) that the top-k
                    # chain may destroy; 1/W_SCALE descale folded into scale
                    u2u = wkpool.tile([128, 3 * E], F32, tag="u2u")
                    nc.scalar.activation(
                        u2u,
                        psum_t,
                        mybir.ActivationFunctionType.Exp,
                        scale=1.0 / W_SCALE,
                    )
                    u2 = u2u[:, :E]
                    u = u2u[:, E : 2 * E]
                    y = u2u[:, 2 * E :]

                    # largest-20 u2 = bottom-20 scores. 2x(max8+replace)
                    # removes 16, then ranks 17-24 -> index 4 = 21st largest
                    # u2 = 21st smallest score = boundary kept expert.
                    r1 = smpool.tile([128, 8], F32, tag="r1")
                    nc.vector.max(r1, y)
                    nc.vector.match_replace(y, r1, y, NEG_BIG)
                    r2 = smpool.tile([128, 8], F32, tag="r2")
                    nc.vector.max(r2, y)
                    nc.vector.match_replace(y, r2, y, NEG_BIG)
                    r3 = smpool.tile([128, 8], F32, tag="r3")
                    nc.vector.max(r3, y)
                    thr2 = r3[:, (N_BOTTOM - 16) : (N_BOTTOM - 16 + 1)]

                    # wm = u * (u2 <= thr2); ws = sum(wm)
                    wm = wkpool.tile([128, E], F32, tag="wm")
                    ws = smpool.tile([128, 1], F32, tag="ws")
                    nc.vector.scalar_tensor_tensor(
                        out=wm,
                        in0=u2,
                        scalar=thr2,
                        in1=u,
                        op0=mybir.AluOpType.is_le,
                        op1=mybir.AluOpType.mult,
                        accum_out=ws,
                    )
                    # og = wm/ws (f16 cast on write); ws -> 1/ws, unused
                    nc.gpsimd.normalize_recip(og[:, s, :], wm, ws)
                out_dmas.append((t0, L, og))

            # software pipeline: selection(g-1) emitted after phase(g) so
            # the in-order PE/Scalar phase streams run ahead at DMA pace
            prev = None
            for gi, (t0, L, xg) in enumerate(xgs):
                ps = emit_mms(t0, L, xg)
                psum_ts = emit_copy_transpose(L, ps)
                if prev is not None:
                    emit_selection(*prev)
                prev = (t0, L, psum_ts)
            emit_selection(*prev)

            # all output DMAs at the very end of the SP stream so no x
            # dispatch ever queues behind an output wait
            for t0, L, og in out_dmas:
                nc.sync.dma_start(
                    out=out_d[t0 : t0 + L, :].rearrange("(s p) e -> p s e", p=128),
                    in_=og,
                )

    nc.finalize()
    return nc


_NC = None
LAST_EXEC_NS = None
LAST_RESULTS = None


def _get_nc():
    global _NC
    if _NC is None:
        _NC = _build_nc()
    return _NC


def _pack_x(shard_t_f16):
    """[D, TPC] f16 -> [JB*128, CPL*TPC] with 4KB-contiguous group lines."""
    x4 = shard_t_f16.reshape(JB, CPL, 128, TPC)  # [j, c, p, t]
    blocks = []
    t0 = 0
    for L in GROUPS:
        blocks.append(
            x4[:, :, :, t0 : t0 + L].transpose(0, 2, 1, 3).reshape(JB, 128, CPL * L)
        )
        t0 += L
    return np.ascontiguousarray(
        np.concatenate(blocks, axis=2).reshape(JB * 128, CPL * TPC)
    )


def _make_mconst():
    m = np.zeros((E, 3 * E), np.float16)
    idx = np.arange(E)
    m[idx, idx] = np.float16(-1.0)
    m[idx, E + idx] = np.float16(1.0)
    m[idx, 2 * E + idx] = np.float16(-1.0)
    return m


def kernel(x, gate_w, trace=False):
    global LAST_EXEC_NS, LAST_RESULTS
    from concourse.bass_utils import run_bass_kernel_spmd

    x = np.asarray(x)
    gate_w = np.asarray(gate_w)
    Btot = x.shape[0] * x.shape[1]
    x2 = x.reshape(Btot, D).astype(np.float32, copy=False)

    wt = np.ascontiguousarray(
        gate_w.astype(np.float32, copy=False).T * np.float32(W_SCALE / TEMPERATURE)
    )
    whl = np.ascontiguousarray(wt.astype(np.float16))
    mconst = _make_mconst()

    nc = _get_nc()
    in_maps = []
    for i in range(N_CORES):
        shard_t = np.ascontiguousarray(
            x2[i * TPC : (i + 1) * TPC].T.astype(np.float16)
        )
        in_maps.append({"xpk": _pack_x(shard_t), "whl": whl, "mconst": mconst})

    kwargs = {}
    if trace:
        try:
            import antenv.axon_hooks  # noqa: F401  (shimmed by test harness)

            kwargs["trace"] = True
        except ImportError:
            pass
    res = run_bass_kernel_spmd(nc, in_maps, core_ids=list(range(N_CORES)), **kwargs)
    LAST_EXEC_NS = res.exec_time_ns
    LAST_RESULTS = res
    out = np.concatenate(
        [res.results[i]["out"].astype(np.float32) for i in range(N_CORES)], axis=0
    )
    return out.reshape(x.shape[0], x.shape[1], E)
